# revision 2
# baseline (speedup 1.0000x reference)
"""Trainium2 Bass kernel for NonLinearSelfAttention.

Computes, per batch b:
    S    = x_b @ x_b.T * C**-0.5          [N, N]
    P    = softmax(S, axis=-1)
    out  = (P @ x_b) @ W.T + bias         [N, OUT]

Sharding: batch-data-parallel, one batch per NeuronCore (8 cores).

Per-core algorithm (N=4096, C=128):
  - E = exp(scale*S) is symmetric, so the tile E[J-block, A-block] computed in
    [j, i] layout is directly the lhsT needed by the P@V matmul for output
    block A — no transposes in the main loop.
  - The Linear folds through the attention: y = (E @ [z | 1]) / r + bias with
    z = x @ W.T, because (P x) W.T = P (x W.T).  The appended ones column
    produces the softmax row-sums r in per-partition layout for free
    (r_i = sum_j E[j, i] = sum_j E[i, j] by symmetry).
  - No max-subtraction needed: logits are ~N(0,1) with diagonal ~ sqrt(C)+,
    max ~ 20, exp(20) is well inside fp32 range.
"""
import numpy as np

import concourse.bass as bass
import concourse.tile as tile
from concourse import bacc, mybir
from concourse import bass_utils
from concourse.masks import make_identity

B = 8          # batches = cores
N = 4096       # sequence length
C = 128        # feature dim
OUT = 128      # linear out dim
NT = N // 128  # 32 j-tiles
QW = 512       # i-columns processed per quad-block
NQ = N // QW   # 8 quad blocks
SCALE = float(C) ** -0.5

F32 = mybir.dt.float32
F32R = mybir.dt.float32r
BF16 = mybir.dt.bfloat16


def _build(ctx_dtype=BF16):
    nc = bacc.Bacc("TRN2", target_bir_lowering=False, debug=False, num_devices=B)
    x_d = nc.dram_tensor("x", [N, C], F32, kind="ExternalInput").ap()
    w_d = nc.dram_tensor("W", [OUT, C], F32, kind="ExternalInput").ap()
    b_d = nc.dram_tensor("b", [OUT], F32, kind="ExternalInput").ap()
    o_d = nc.dram_tensor("out", [N, OUT], F32, kind="ExternalOutput").ap()

    with tile.TileContext(nc) as tc:
        with tc.tile_pool(name="const", bufs=1) as const, \
             tc.tile_pool(name="bwork", bufs=3) as bwork, \
             tc.tile_pool(name="ywork", bufs=4) as ywork, \
             tc.tile_pool(name="ps_work", bufs=3, space="PSUM") as ps_work, \
             tc.tile_pool(name="ps_acc", bufs=4, space="PSUM") as ps_acc:

            # ---- setup ----
            ident = const.tile([128, 128], F32)
            make_identity(nc, ident)

            x_nat = const.tile([128, NT, 128], F32)       # x tiles [j within tile, c]
            for j in range(NT):
                nc.sync.dma_start(x_nat[:, j, :], x_d[j * 128:(j + 1) * 128, :])

            w_sb = const.tile([128, 128], F32)            # W [o, c]
            nc.sync.dma_start(w_sb, w_d)
            bias_bc = const.tile([128, 128], F32)         # bias broadcast to all partitions
            nc.sync.dma_start(bias_bc, bass.AP(tensor=b_d.tensor, offset=b_d.offset,
                                               ap=[[0, 128]] + b_d.ap))

            # xT (f32r) via PE transpose
            xT = const.tile([128, N], F32R)               # [c, n]
            for j in range(NT):
                t_ps = ps_work.tile([128, 512], F32, name="t_ps", tag="pswork")
                nc.tensor.transpose(t_ps[:, 0:128], x_nat[:, j, :], ident)
                nc.vector.tensor_copy(xT[:, j * 128:(j + 1) * 128], t_ps[:, 0:128])

            # wT (f32r): wT[c, o] = W[o, c]
            wt_ps = ps_work.tile([128, 512], F32, name="t_ps", tag="pswork")
            nc.tensor.transpose(wt_ps[:, 0:128], w_sb, ident)
            wT = const.tile([128, 128], F32R)
            nc.vector.tensor_copy(wT, wt_ps[:, 0:128])

            # z~ = [x @ W.T | 1]  (bf16), tiled [j within tile, 129]
            zt = const.tile([128, NT, 129], ctx_dtype)
            nc.vector.memset(zt[:, :, 128], 1.0)
            for j in range(NT):
                z_ps = ps_work.tile([128, 512], F32, name="z_ps", tag="pswork")
                nc.tensor.matmul(z_ps[:, 0:128], xT[:, j * 128:(j + 1) * 128], wT,
                                 start=True, stop=True)
                nc.vector.tensor_copy(zt[:, j, 0:128], z_ps[:, 0:128])

            # ---- main loop ----
            for q in range(NQ):
                acc = [ps_acc.tile([128, 129], F32, name=f"acc{k}", tag="acc")
                       for k in range(QW // 128)]
                for j in range(NT):
                    s_ps = ps_work.tile([128, QW], F32, name="s_ps", tag="pswork")
                    nc.tensor.matmul(s_ps, xT[:, j * 128:(j + 1) * 128],
                                     xT[:, q * QW:(q + 1) * QW], start=True, stop=True)
                    b_sb = bwork.tile([128, QW], ctx_dtype)
                    nc.scalar.activation(b_sb, s_ps, mybir.ActivationFunctionType.Exp,
                                         scale=SCALE)
                    for k in range(QW // 128):
                        nc.tensor.matmul(acc[k], b_sb[:, k * 128:(k + 1) * 128],
                                         zt[:, j, :], start=(j == 0), stop=(j == NT - 1))
                # epilogue: y = acc[:, :128] / acc[:, 128] + bias
                for k in range(QW // 128):
                    rinv = ywork.tile([128, 1], F32)
                    nc.vector.reciprocal(rinv, acc[k][:, 128:129])
                    y_sb = ywork.tile([128, 128], F32)
                    nc.vector.scalar_tensor_tensor(
                        y_sb, acc[k][:, 0:128], rinv, bias_bc,
                        op0=mybir.AluOpType.mult, op1=mybir.AluOpType.add)
                    blk = q * (QW // 128) + k
                    nc.sync.dma_start(o_d[blk * 128:(blk + 1) * 128, :], y_sb)

    nc.compile()
    return nc


_NC_CACHE = {}


def _get_nc():
    if "nc" not in _NC_CACHE:
        _NC_CACHE["nc"] = _build()
    return _NC_CACHE["nc"]


def kernel(x, W, b, _trace=False):
    """x: [8, 4096, 128] f32, W: [128, 128] f32, b: [128] f32 -> [8, 4096, 128] f32."""
    nc = _get_nc()
    x = np.ascontiguousarray(np.asarray(x, dtype=np.float32))
    W = np.ascontiguousarray(np.asarray(W, dtype=np.float32))
    b = np.ascontiguousarray(np.asarray(b, dtype=np.float32))
    in_maps = [{"x": x[i], "W": W, "b": b} for i in range(B)]
    res = bass_utils.run_bass_kernel_spmd(nc, in_maps, core_ids=list(range(B)),
                                          trace=_trace)
    out = np.stack([r["out"] for r in res.results]).astype(np.float32)
    if _trace:
        return out, res
    return out
